# revision 9
# baseline (speedup 1.0000x reference)
"""GroupedQueryAttention Trainium2 Bass kernel (v3, fused chunk pipeline).

Sharding: 8 cores = 2 (batch) x 4 (KV groups). Each core computes, for its
(b, g): q/k/v projections for the group's 4 query heads + 1 kv head, causal
attention, and the partial output projection ctx_g @ Wo[g-rows]. Host sums
the 4 group partials per batch and adds the bias.

v3 changes vs v2:
- 256-query chunks with pair-packed PSUM banks (2 heads per bank), so the
  whole working set is 8 banks: proj 3 + scores 2 + ctx 2 + rowsum 1.
- Projection matmuls for chunk c+1 are interleaved into attention of chunk
  c at emission, so the PE never idles waiting for exp (ACT) results and
  stays at its ramped clock.
- Causal mask applied on the PE (ident @ negmask accumulate) instead of DVE,
  removing DVE from the S->exp critical path.
- Softmax normalization deferred to the output-projection phase (bc matmul +
  one DVE multiply per head pair); attention epilogue only evicts raw ctx.
- Output projection overlaps the tail of attention; its PSUM tiles rotate
  through the (then idle) projection banks.
"""
import sys
sys.path.insert(0, '/opt/trn_rl_repo')

import numpy as np
import ml_dtypes
import concourse.bass as bass
import concourse.bacc as bacc
import concourse.tile as tile
import concourse.mybir as mybir
from concourse import bass_utils
from concourse.bass_interp import get_hw_module
from contextlib import ExitStack, nullcontext

F32 = mybir.dt.float32
F32R = mybir.dt.float32r
BF16 = mybir.dt.bfloat16
AF = mybir.ActivationFunctionType
ALU = mybir.AluOpType
BF = ml_dtypes.bfloat16

SEQ = 2048
D = 2048
HD = 128          # head dim
NHL = 4           # query heads per core (group size)
QC = 256          # query chunk
NCH = SEQ // QC   # 8 chunks
NDT = D // 128    # 16 contraction tiles
SCALE = 1.0 / float(np.sqrt(HD))
NEG = -1e30
XCH = NDT * QC    # x-chunk columns (4096)


def build_program(niter=1, evict_pool=False, no_attn=False,
                  no_pb=False, nab_override=None, bc_base0=True,
                  no_strided_exp=False):
    nc = bacc.Bacc("TRN2", target_bir_lowering=False, debug=False,
                   enable_asserts=False, num_devices=8)
    # XT[p, c*4096 + t*256 + j] = x[c*256+j, t*128+p]   (chunk-major)
    XT = nc.dram_tensor("XT", [128, NCH * XCH], BF16, kind="ExternalInput").ap()
    # WC[p, t*768+col] = concat(Wq_g|Wk_g|Wv_g)[t*128+p, col]
    WC = nc.dram_tensor("WC", [128, NDT * 768], BF16, kind="ExternalInput").ap()
    # WO[p, s*2048+j] = Wo[g*512+s*128+p, j]
    WO = nc.dram_tensor("WO", [128, NHL * D], BF16, kind="ExternalInput").ap()
    NM = nc.dram_tensor("NM", [128, 128], BF16, kind="ExternalInput").ap()
    ONESC = nc.dram_tensor("ONESC", [128, 1], BF16, kind="ExternalInput").ap()
    ONESR = nc.dram_tensor("ONESR", [128, 128], F32R, kind="ExternalInput").ap()
    IDENT = nc.dram_tensor("IDENT", [128, 128], BF16, kind="ExternalInput").ap()
    # OUT[p, mb*2048+j] = out[mb*128+p, j]
    OUT = nc.dram_tensor("out", [128, (SEQ // 128) * D], BF16,
                         kind="ExternalOutput").ap()

    if evict_pool:
        EV = lambda dst, src: nc.gpsimd.tensor_copy(dst, src)
    else:
        EV = lambda dst, src: nc.vector.tensor_copy(dst, src)

    with tile.TileContext(nc) as tc:
        with (tc.For_i(0, niter, 1) if niter > 1 else nullcontext()):
          with ExitStack() as octx:
            const = octx.enter_context(tc.tile_pool(name="const", bufs=1))
            resid = octx.enter_context(tc.tile_pool(name="resid", bufs=1))
            xp = octx.enter_context(tc.tile_pool(name="xp", bufs=2))
            qtp = octx.enter_context(tc.tile_pool(name="qtp", bufs=2))
            vtp = octx.enter_context(tc.tile_pool(name="vtp", bufs=2))
            estp = octx.enter_context(tc.tile_pool(name="estp", bufs=4))
            outp = octx.enter_context(tc.tile_pool(name="outp", bufs=2))
            projp = octx.enter_context(
                tc.tile_pool(name="projp", bufs=3, space="PSUM"))
            stp = octx.enter_context(
                tc.tile_pool(name="stp", bufs=2, space="PSUM"))
            ctxps = octx.enter_context(
                tc.tile_pool(name="ctxps", bufs=2, space="PSUM"))
            rsp = octx.enter_context(
                tc.tile_pool(name="rsp", bufs=1, space="PSUM"))

            negmask = const.tile([128, 128], BF16, name="negmask", tag="nm")
            onesc = const.tile([128, 1], BF16, name="onesc", tag="oc")
            onesr = const.tile([128, 128], F32R, name="onesr", tag="or")
            ident = const.tile([128, 128], BF16, name="ident", tag="id")

            wc = resid.tile([128, NDT * 768], BF16, name="wc", tag="wc")
            wo = resid.tile([128, NHL * D], BF16, name="wo", tag="wo")
            kT = resid.tile([128, SEQ], BF16, name="kT", tag="kT")
            vcat = resid.tile([128, SEQ], BF16, name="vcat", tag="vcat")
            # raw (unnormalized) ctx, head-major: [hd, h*2048 + q]
            ctxr = resid.tile([128, NHL * SEQ], BF16, name="ctxr", tag="ctxr")
            # reciprocal row-sums, head h at partition 32*(h%2),
            # column block (h//2)*SEQ  (all at partition 0 if bc_base0)
            recips = resid.tile([128, (NHL if bc_base0 else 2) * SEQ], F32R,
                                name="recips", tag="recips")

            def rloc(h):
                if bc_base0:
                    return 0, h * SEQ
                return 32 * (h % 2), (h // 2) * SEQ

            x_t = [None] * NCH
            qt_t = [None] * NCH

            def xdma(c, parts=1):
                x_t[c] = xp.tile([128, XCH], BF16, name=f"x{c}", tag="x")
                w = XCH // parts
                for q in range(parts):
                    nc.sync.dma_start(
                        x_t[c][:, q * w:(q + 1) * w],
                        XT[:, c * XCH + q * w: c * XCH + (q + 1) * w])

            # ---- initial DMAs: wc quarters interleaved with x(0) quarters
            x_t[0] = xp.tile([128, XCH], BF16, name="x0", tag="x")
            WQ = NDT * 768 // 4
            XQ = XCH // 4
            for q in range(4):
                nc.sync.dma_start(wc[:, q * WQ:(q + 1) * WQ],
                                  WC[:, q * WQ:(q + 1) * WQ])
                nc.sync.dma_start(x_t[0][:, q * XQ:(q + 1) * XQ],
                                  XT[:, q * XQ:(q + 1) * XQ])
            nc.sync.dma_start(negmask[:], NM[:, :])
            nc.sync.dma_start(onesc[:], ONESC[:, :])
            nc.sync.dma_start(onesr[:], ONESR[:, :])
            nc.sync.dma_start(ident[:], IDENT[:, :])

            # ---------------- proj units ----------------
            def proj_units(c):
                """Emit-ready closures for chunk c's projections."""
                psqA = projp.tile([128, 2 * QC], F32, name=f"psqA{c}",
                                  tag="proj")
                psqB = projp.tile([128, 2 * QC], F32, name=f"psqB{c}",
                                  tag="proj")
                pskv = projp.tile([128, 2 * QC], F32, name=f"pskv{c}",
                                  tag="proj")

                def unit(t):
                    xs = x_t[c][:, t * QC:(t + 1) * QC]
                    first = (t == 0)
                    last = (t == NDT - 1)
                    for hh in (0, 1):
                        nc.tensor.matmul(
                            psqA[:, hh * QC:(hh + 1) * QC],
                            wc[:, t * 768 + hh * 128: t * 768 + (hh + 1) * 128],
                            xs, start=(first and hh == 0),
                            stop=(last and hh == 1))
                    for hh in (0, 1):
                        nc.tensor.matmul(
                            psqB[:, hh * QC:(hh + 1) * QC],
                            wc[:, t * 768 + (2 + hh) * 128:
                               t * 768 + (3 + hh) * 128],
                            xs, start=(first and hh == 0),
                            stop=(last and hh == 1))
                    nc.tensor.matmul(pskv[:, 0:QC],
                                     wc[:, t * 768 + 512: t * 768 + 640],
                                     xs, start=first, stop=False)
                    nc.tensor.matmul(pskv[:, QC:2 * QC],
                                     wc[:, t * 768 + 640: t * 768 + 768],
                                     xs, start=False, stop=last)

                vt_box = [None]

                def evicts():
                    qt_t[c] = qtp.tile([128, NHL * QC], BF16, name=f"qt{c}",
                                       tag="qt")
                    EV(qt_t[c][:, 0:2 * QC], psqA[:])
                    EV(qt_t[c][:, 2 * QC:4 * QC], psqB[:])
                    EV(kT[:, c * QC:(c + 1) * QC], pskv[:, 0:QC])
                    vt_box[0] = vtp.tile([128, QC], BF16, name=f"vt{c}",
                                         tag="vt")
                    EV(vt_box[0][:], pskv[:, QC:2 * QC])

                def transp(t2):
                    trp = stp.tile([128, 128], BF16, name=f"trp{c}_{t2}",
                                   tag="st")
                    nc.tensor.transpose(trp[:], vt_box[0][:, t2 * 128:
                                                          (t2 + 1) * 128],
                                        ident[:])
                    EV(vcat[:, c * QC + t2 * 128: c * QC + (t2 + 1) * 128],
                       trp[:])

                units = []
                for t in range(NDT):
                    units.append(lambda t=t: unit(t))
                    if t == 3 and c + 1 < NCH:
                        units.append(lambda c=c: xdma(c + 1))
                    if t == 7 and c == 0:
                        units.append(
                            lambda: nc.sync.dma_start(wo[:], WO[:, :]))
                units.append(evicts)
                units.append(lambda: transp(0))
                units.append(lambda: transp(1))
                return units

            # ---------------- attention ----------------
            ctx_banks = [None, None]
            rs_box = [None]

            def emit_pv(c, ki, p, est, j, last):
                vs = vcat[:, ki * 128:(ki + 1) * 128]
                ctb = ctx_banks[p]
                rs_row = rs_box[0][32 * p:32 * p + 1, :]
                if j == 1:
                    for hh in (0, 1):
                        nc.tensor.matmul(
                            ctb[:, hh * QC + 128:(hh + 1) * QC], vs,
                            est[:, hh * QC + 128:(hh + 1) * QC],
                            start=False, stop=(last and hh == 1))
                    for hh in (0, 1):
                        nc.tensor.matmul(
                            rs_row[:, hh * QC + 128:(hh + 1) * QC], onesc[:],
                            est[:, hh * QC + 128:(hh + 1) * QC],
                            start=False, stop=(last and hh == 1))
                else:
                    nc.tensor.matmul(ctb[:], vs, est[:],
                                     start=(ki == 0), stop=False)
                    nc.tensor.matmul(rs_row[:], onesc[:], est[:],
                                     start=(ki == 0), stop=False)

            def emit_attn(c, punits):
                nki = 2 * (c + 1)
                pairs = [(ki, p) for ki in range(nki) for p in (0, 1)]
                npr = len(pairs)
                ctx_banks[0] = ctxps.tile([128, 2 * QC], F32,
                                          name=f"ctx{c}_0", tag="ctx")
                ctx_banks[1] = ctxps.tile([128, 2 * QC], F32,
                                          name=f"ctx{c}_1", tag="ctx")
                rs_box[0] = rsp.tile([128, 2 * QC], F32, name=f"rs{c}",
                                     tag="rs")
                emitted = 0
                pending = None
                for i, (ki, p) in enumerate(pairs):
                    j = ki - 2 * c
                    qb = p * 2 * QC
                    qt = qt_t[c]
                    stt = stp.tile([128, 2 * QC], F32,
                                   name=f"st{c}_{ki}_{p}", tag="st")
                    est = estp.tile([128, 2 * QC], BF16,
                                    name=f"est{c}_{ki}_{p}", tag="est")
                    ks = kT[:, ki * 128:(ki + 1) * 128]
                    if j < 0:
                        nc.tensor.matmul(stt[:], ks, qt[:, qb:qb + 2 * QC],
                                         start=True, stop=True)
                        nc.scalar.activation(est[:], stt[:], AF.Exp,
                                             scale=SCALE)
                    elif j == 0:
                        nc.tensor.matmul(stt[:], ks, qt[:, qb:qb + 2 * QC],
                                         start=True, stop=False)
                        nc.tensor.matmul(stt[:, 0:128], ident[:], negmask[:],
                                         start=False, stop=False)
                        nc.tensor.matmul(stt[:, QC:QC + 128], ident[:],
                                         negmask[:], start=False, stop=True)
                        nc.scalar.activation(est[:], stt[:], AF.Exp,
                                             scale=SCALE)
                    else:  # j == 1: only the upper 128 queries of each head
                        nc.tensor.matmul(stt[:, 128:QC], ks,
                                         qt[:, qb + 128:qb + QC],
                                         start=True, stop=False)
                        nc.tensor.matmul(stt[:, QC + 128:2 * QC], ks,
                                         qt[:, qb + QC + 128:qb + 2 * QC],
                                         start=False, stop=False)
                        nc.tensor.matmul(stt[:, 128:QC], ident[:], negmask[:],
                                         start=False, stop=False)
                        nc.tensor.matmul(stt[:, QC + 128:2 * QC], ident[:],
                                         negmask[:], start=False, stop=True)
                        if no_strided_exp:
                            for hh in (0, 1):
                                nc.scalar.activation(
                                    est[:, hh * QC + 128:(hh + 1) * QC],
                                    stt[:, hh * QC + 128:(hh + 1) * QC],
                                    AF.Exp, scale=SCALE)
                        else:
                            sv = stt[:].rearrange("p (h w) -> p h w",
                                                  h=2)[:, :, 128:QC]
                            ev = est[:].rearrange("p (h w) -> p h w",
                                                  h=2)[:, :, 128:QC]
                            nc.scalar.activation(ev, sv, AF.Exp, scale=SCALE)
                    want = ((i + 1) * len(punits)) // npr
                    while emitted < want:
                        punits[emitted]()
                        emitted += 1
                    if pending is not None:
                        emit_pv(c, *pending)
                    pending = (ki, p, est, j, ki == nki - 1)
                emit_pv(c, *pending)
                while emitted < len(punits):
                    punits[emitted]()
                    emitted += 1
                # epilogue: reciprocals of row sums + raw ctx eviction
                with nc.allow_low_precision(reason="fp32r recip"):
                    for h in range(NHL):
                        pr, hh = h // 2, h % 2
                        rp, rcol = rloc(h)
                        nc.vector.reciprocal(
                            recips[rp:rp + 1,
                                   rcol + c * QC: rcol + (c + 1) * QC],
                            rs_box[0][32 * pr:32 * pr + 1,
                                      hh * QC:(hh + 1) * QC])
                for p in (0, 1):
                    src = ctx_banks[p][:].rearrange("p (h n) -> p h n", h=2)
                    dst = ctxr[:].rearrange("p (h n) -> p h n",
                                            h=NHL)[:, 2 * p:2 * p + 2,
                                                   c * QC:(c + 1) * QC]
                    EV(dst, src)

            # ---------------- phase B units (normalize + out-proj) --------
            bcs = {}
            otile = {}

            def pb_units():
                units = []

                def bc_unit(c):
                    bcs[c] = []
                    for p in (0, 1):
                        bcp = projp.tile([128, 2 * QC], F32,
                                         name=f"bc{c}_{p}", tag="proj")
                        for hh in (0, 1):
                            h = 2 * p + hh
                            rp, rcol = rloc(h)
                            nc.tensor.matmul(
                                bcp[:, hh * QC:(hh + 1) * QC],
                                onesr[rp:rp + 1, :],
                                recips[rp:rp + 1,
                                       rcol + c * QC: rcol + (c + 1) * QC],
                                start=(hh == 0), stop=(hh == 1))
                        bcs[c].append(bcp)

                def mult_unit(c):
                    for p in (0, 1):
                        v = ctxr[:].rearrange("p (h n) -> p h n",
                                              h=NHL)[:, 2 * p:2 * p + 2,
                                                     c * QC:(c + 1) * QC]
                        nc.vector.tensor_tensor(
                            v, v,
                            bcs[c][p][:].rearrange("p (h n) -> p h n", h=2),
                            ALU.mult)

                def fill(mb, n2):
                    if n2 == 0:
                        otile[mb] = outp.tile([128, D], BF16,
                                              name=f"ot{mb}", tag="ot")
                    pso = projp.tile([128, 512], F32, name=f"pso{mb}_{n2}",
                                     tag="proj")
                    for s in range(NHL):
                        nc.tensor.matmul(
                            pso[:],
                            ctxr[:, s * SEQ + mb * 128: s * SEQ + mb * 128
                                 + 128],
                            wo[:, s * D + n2 * 512: s * D + (n2 + 1) * 512],
                            start=(s == 0), stop=(s == NHL - 1))
                    if n2 % 2 == 0:
                        nc.vector.tensor_copy(
                            otile[mb][:, n2 * 512:(n2 + 1) * 512], pso[:])
                    else:
                        nc.scalar.copy(
                            otile[mb][:, n2 * 512:(n2 + 1) * 512], pso[:])

                def op_units(c):
                    res = []
                    for m in (0, 1):
                        mb = 2 * c + m
                        for n2 in range(4):
                            res.append(lambda mb=mb, n2=n2: fill(mb, n2))
                        res.append(lambda mb=mb: nc.sync.dma_start(
                            OUT[:, mb * D:(mb + 1) * D], otile[mb][:]))
                    return res

                safe = 0
                for c in range(NCH):
                    if c == NCH - 1:
                        safe = len(units)  # bc(7) needs recips(7): not safe
                    units.append(lambda c=c: bc_unit(c))
                    units.append(lambda c=c: mult_unit(c))
                    if c >= 1:
                        units += op_units(c - 1)
                units += op_units(NCH - 2)
                units += op_units(NCH - 1)
                return units, safe

            # ---------------- main schedule ----------------
            for u in proj_units(0):
                u()
            if no_attn:
                for c in range(NCH - 1):
                    for u in proj_units(c + 1):
                        u()
                for mb in range(SEQ // 128):
                    ot = outp.tile([128, D], BF16, name=f"otz{mb}", tag="ot")
                    nc.vector.memset(ot[:], 0.0)
                    nc.sync.dma_start(OUT[:, mb * D:(mb + 1) * D], ot[:])
            else:
                for c in range(NCH - 1):
                    emit_attn(c, proj_units(c + 1))
                if no_pb:
                    emit_attn(NCH - 1, [])
                    for mb in range(SEQ // 128):
                        ot = outp.tile([128, D], BF16, name=f"otz{mb}",
                                       tag="ot")
                        nc.vector.memset(ot[:], 0.0)
                        nc.sync.dma_start(OUT[:, mb * D:(mb + 1) * D], ot[:])
                else:
                    pbu, safe = pb_units()
                    # attention of last chunk absorbs safe phase-B prefix
                    nab = min(safe, 2 * (2 * NCH))
                    if nab_override is not None:
                        nab = min(nab, nab_override)
                    emit_attn(NCH - 1, pbu[:nab])
                    for u in pbu[nab:]:
                        u()

    nc.compile()
    nc.m = get_hw_module(nc.m)
    return nc


_NC = None


def _get_nc():
    global _NC
    if _NC is None:
        _NC = build_program()
    return _NC


def _consts():
    negmask = np.where(np.arange(128)[:, None] <= np.arange(128)[None, :],
                       0.0, NEG).astype(np.float32)
    return {
        "NM": negmask.astype(BF),
        "ONESC": np.ones((128, 1), BF),
        "ONESR": np.ones((128, 128), np.float32),
        "IDENT": np.eye(128, dtype=BF),
    }


def _pack(a, ntile):
    """[ntile*128, N] f32 -> [128, ntile*N] bf16 with tile t at cols t*N."""
    n = a.shape[1]
    return np.ascontiguousarray(
        a.reshape(ntile, 128, n).transpose(1, 0, 2).reshape(128, ntile * n)
    ).astype(BF)


def _pack_x(xb):
    """[2048, 2048] f32 -> [128, 32768] bf16 chunk-major:
    XT[p, c*4096 + t*256 + j] = x[c*256+j, t*128+p]."""
    return np.ascontiguousarray(
        xb.reshape(NCH, QC, NDT, 128).transpose(3, 0, 2, 1)
        .reshape(128, NCH * XCH)).astype(BF)


def make_in_maps(x, Wq, Wk, Wv, Wo):
    consts = _consts()
    in_maps = []
    xTp = [None, None]
    for i in range(8):
        bi, g = i // 4, i % 4
        if xTp[bi] is None:
            xTp[bi] = _pack_x(np.asarray(x[bi], np.float32))
        wcat = np.concatenate([Wq[:, g * 512:(g + 1) * 512],
                               Wk[:, g * 128:(g + 1) * 128],
                               Wv[:, g * 128:(g + 1) * 128]], axis=1)
        in_maps.append({
            "XT": xTp[bi],
            "WC": _pack(wcat, NDT),
            "WO": _pack(np.ascontiguousarray(Wo[g * 512:(g + 1) * 512, :]),
                        NHL),
            **consts,
        })
    return in_maps


def _unpack_out(o):
    """[128, 16*2048] bf16 -> [2048, 2048] f32."""
    return o.reshape(128, SEQ // 128, D).transpose(1, 0, 2).reshape(
        SEQ, D).astype(np.float32)


def kernel(x, Wq, Wk, Wv, Wo, bo):
    x = np.asarray(x, np.float32)
    Wq = np.asarray(Wq, np.float32)
    Wk = np.asarray(Wk, np.float32)
    Wv = np.asarray(Wv, np.float32)
    Wo = np.asarray(Wo, np.float32)
    bo = np.asarray(bo, np.float32)
    b = x.shape[0]
    nc = _get_nc()
    in_maps = make_in_maps(x, Wq, Wk, Wv, Wo)
    res = bass_utils.run_bass_kernel_spmd(nc, in_maps,
                                          core_ids=list(range(8)),
                                          trace=False)
    out = np.zeros((b, SEQ, D), np.float32)
    for i in range(8):
        bi = i // 4
        out[bi] += _unpack_out(np.asarray(res.results[i]["out"]))
    out += bo[None, None, :]
    return out
